# revision 8
# baseline (speedup 1.0000x reference)
"""Axial attention block (B=8, C=512, H=W=128, 8 heads) on 8 Trainium2 cores.

Sharding: data-parallel over batch — one batch element per NeuronCore. Each
core runs both axial passes on its (C, H, W) slice and produces the full
residual sum xs + oh + ow.

Pass structure (all DMA contiguous-run; no strided DRAM access):
  - Pass 1 (HEIGHT attention, sequences along h, one per w): reads xtbf
    (C,W,H) chunks, computes oh tiles in (c, w-chunk, h) layout and writes
    them to a block-tiled DRAM scratch ohT2[hb][c, w, hi] (h = hb*8 + hi).
    The SBUF stage tile is laid out (hb, w, hi) so both DMA sides have
    >=32B/512B contiguous runs.
  - Pass 2 (WIDTH attention, sequences along w, one per h): h-chunk hb reads
    xbf + xs(f32) chunks and the matching ohT2[hb] block (contiguous), folds
    oh into the f32 residual once per chunk (GpSimd), then out = ow + resid
    per group (VectorE) and writes natural-layout output.

Matmul inputs are pre-cast to bf16 on the host (xbf natural, xtbf h<->w
swapped); weights pre-transposed to (c_in, c_out) bf16.

Per-sequence attention (S=128, dh=64): scoresT = K^T.T @ Q^T per head in
(s_k, s_q) layout, parity-split over two PSUM banks (concurrent row-group
matmuls must not share a bank); exp on ScalarE (max-subtraction skipped —
scaled scores are bounded ~7); denominators via ones-matmul on TensorE
(replicated rows); reciprocal_approx_fast + normalize on VectorE; AV packs
all 8 heads into one PSUM bank in (c, s_q) layout; O-projection batched
over 4 sequences.
"""
import os
import numpy as np
import ml_dtypes

P = 128          # partitions
C = 512          # channels
S = 128          # sequence length (H and W)
NCB = C // P     # channel blocks
NH = 8           # heads
DH = C // NH     # head dim
G = 4            # sequences per projection group
HC1 = 16         # w-chunk, height pass
HC2 = 8          # h-chunk, width pass (= hi block size of ohT2)
HB = S // HC2    # number of h blocks
NCORES = 8

_BF16 = ml_dtypes.bfloat16

# schedule-tuning knobs (env-overridable for experiments)
PROJ_BUFS = int(os.environ.get("K_PROJ_BUFS", "2"))
ATTN_BUFS = int(os.environ.get("K_ATTN_BUFS", "2"))
ET_BUFS = int(os.environ.get("K_ET_BUFS", "2"))
QK_BUFS = int(os.environ.get("K_QK_BUFS", "2"))
VT_BUFS = int(os.environ.get("K_VT_BUFS", "2"))
OT_BUFS = int(os.environ.get("K_OT_BUFS", "2"))
RR_BUFS = int(os.environ.get("K_RR_BUFS", "2"))
PO_BUFS = int(os.environ.get("K_PO_BUFS", "2"))

_PROG = None  # cached compiled Bass program


def _build_program():
    from contextlib import ExitStack
    import concourse.tile as tile
    from concourse import bacc, mybir

    f32 = mybir.dt.float32
    bf = mybir.dt.bfloat16
    Exp = mybir.ActivationFunctionType.Exp

    nc = bacc.Bacc("TRN2", target_bir_lowering=False, debug=False)

    xf = nc.dram_tensor("xf", [C, S, S], f32, kind="ExternalInput").ap()
    xbf = nc.dram_tensor("xbf", [C, S, S], bf, kind="ExternalInput").ap()
    xtbf = nc.dram_tensor("xtbf", [C, S, S], bf, kind="ExternalInput").ap()
    wnames = ["wq_w", "wk_w", "wv_w", "wo_w", "wq_h", "wk_h", "wv_h", "wo_h"]
    wt = {n: nc.dram_tensor(n, [C, C], bf, kind="ExternalInput").ap() for n in wnames}
    ohT2 = nc.dram_tensor("ohT2", [HB, C, S, HC2], f32).ap()
    out = nc.dram_tensor("out", [C, S, S], f32, kind="ExternalOutput").ap()

    with tile.TileContext(nc) as tc, ExitStack() as topctx:
        const = topctx.enter_context(tc.tile_pool(name="const", bufs=1))

        w_sb = {}
        for n in wnames:
            tiles = []
            for ci in range(NCB):
                t = const.tile([P, C], bf, tag=f"w_{n}_{ci}", name=f"w_{n}_{ci}")
                nc.sync.dma_start(out=t, in_=wt[n][ci * P:(ci + 1) * P, :])
                tiles.append(t)
            w_sb[n] = tiles
        ones_sb = const.tile([P, P], bf, tag="ones", name="ones")
        nc.vector.memset(ones_sb, 1.0)

        def attn_group(src_t, gsl, s0, wq, wk, wv, wo, pools):
            """One group of G sequences -> psum tiles of out-projection
            results, one (P, G*S) tile per c_out block."""
            qk_pool, vt_pool, ot_pool, et_pool, rr_pool, proj_ps, attn_ps, po_ps = pools

            qt_sb, kt_sb = [], []
            for wmat, dst_list, nm in ((wq, qt_sb, "qt"), (wk, kt_sb, "kt")):
                for co in range(NCB):
                    pp = proj_ps.tile([P, G * S], f32, tag="proj", name="pp")
                    for ci in range(NCB):
                        nc.tensor.matmul(
                            pp,
                            lhsT=wmat[ci][:, co * P:(co + 1) * P],
                            rhs=src_t[ci][:, gsl, :],
                            start=(ci == 0), stop=(ci == NCB - 1))
                    sb_t = qk_pool.tile([P, G * S], bf, tag=f"{nm}{co}", name=f"{nm}{co}")
                    nc.scalar.copy(sb_t, pp)
                    dst_list.append(sb_t)

            vt_sb = []
            for sq in range(G):
                pv = proj_ps.tile([P, C], f32, tag="proj", name="pv")
                for ci in range(NCB):
                    nc.tensor.matmul(
                        pv, lhsT=src_t[ci][:, s0 + sq, :], rhs=wv[ci],
                        start=(ci == 0), stop=(ci == NCB - 1))
                vt = vt_pool.tile([P, C], bf, tag=f"vt{sq}", name=f"vt{sq}")
                nc.vector.tensor_copy(vt, pv)
                vt_sb.append(vt)

            ot_full = ot_pool.tile([P, NCB, G * S], bf, tag="ot", name="ot")
            for sq in range(G):
                ssl = slice(sq * S, (sq + 1) * S)
                # scoresT: head h -> col h//2*128 of half (h%2); the two
                # 512-col halves are separate PSUM banks, so even (row-group
                # 0-63) and odd (64-127) head matmuls never share a bank
                st2 = attn_ps.tile([P, 1024], f32, tag="attn", name="st2")
                for h in range(NH):
                    par, cb = h % 2, h // 2
                    rows = slice(par * DH, (par + 1) * DH)
                    nc.tensor.matmul(
                        st2[:, par * 512 + cb * S:par * 512 + (cb + 1) * S],
                        lhsT=kt_sb[h // 2][rows, ssl],
                        rhs=qt_sb[h // 2][rows, ssl],
                        start=True, stop=True)
                et = et_pool.tile([P, 1024], bf, tag="et", name="et")
                nc.scalar.activation(out=et, in_=st2, func=Exp, scale=DH ** -0.5)
                r2 = attn_ps.tile([P, 1024], f32, tag="attn", name="r2")
                nc.tensor.matmul(r2[:, 0:512], lhsT=ones_sb, rhs=et[:, 0:512],
                                 start=True, stop=True)
                nc.tensor.matmul(r2[:, 512:1024], lhsT=ones_sb, rhs=et[:, 512:1024],
                                 start=True, stop=True)
                rr = rr_pool.tile([P, 1024], f32, tag="rr", name="rr")
                nc.vector.reciprocal_approx_fast(out=rr, in_=r2)
                # AV on unnormalized exp; softmax denominators commute past
                # the matmul (pure column scaling), so recip runs on VectorE
                # in parallel with AV on TensorE and the normalize fuses into
                # the psum->sbuf evacuation below.
                po = po_ps.tile([P, 512], f32, tag="po", name="po")
                for h in range(NH):
                    par, cb = h % 2, h // 2
                    nc.tensor.matmul(
                        po[par * DH:(par + 1) * DH, cb * S:(cb + 1) * S],
                        lhsT=vt_sb[sq][:, h * DH:(h + 1) * DH],
                        rhs=et[:, par * 512 + cb * S:par * 512 + (cb + 1) * S],
                        start=True, stop=True)
                # row-half r of po holds heads with parity r; its per-element
                # normalizer is exactly rr[:, r*512:] (rows replicated)
                nc.vector.tensor_mul(
                    ot_full[0:DH, :, ssl],
                    po[0:DH, :].rearrange("p (c s) -> p c s", c=NCB),
                    rr[0:DH, 0:512].rearrange("p (c s) -> p c s", c=NCB))
                nc.vector.tensor_mul(
                    ot_full[DH:P, :, ssl],
                    po[DH:P, :].rearrange("p (c s) -> p c s", c=NCB),
                    rr[DH:P, 512:1024].rearrange("p (c s) -> p c s", c=NCB))
            # O-projection outputs go through the attn pool's 2-bank tiles
            # (pairs of c_out blocks in the two bank halves) so the proj pool
            # frees up for the next group's Q/K/V immediately
            pods = []
            for cop in range(NCB // 2):
                pp2 = attn_ps.tile([P, 1024], f32, tag="attn", name="pp2")
                for half in range(2):
                    co = cop * 2 + half
                    dst = pp2[:, half * 512:(half + 1) * 512]
                    for ci in range(NCB):
                        nc.tensor.matmul(
                            dst,
                            lhsT=wo[ci][:, co * P:(co + 1) * P],
                            rhs=ot_full[:, ci, :],
                            start=(ci == 0), stop=(ci == NCB - 1))
                    pods.append(dst)
            return pods

        def height_pass():
            """Pass 1: height attention (seq along h, one per w).  Writes oh
            to the blocked scratch ohT2[hb][c, w, hi]."""
            wq, wk, wv, wo = (w_sb["wq_h"], w_sb["wk_h"], w_sb["wv_h"], w_sb["wo_h"])
            with ExitStack() as ctx:
                src_pool = ctx.enter_context(tc.tile_pool(name="src1", bufs=2))
                stage_pool = ctx.enter_context(tc.tile_pool(name="stg1", bufs=2))
                qk_pool = ctx.enter_context(tc.tile_pool(name="qk1", bufs=QK_BUFS))
                vt_pool = ctx.enter_context(tc.tile_pool(name="vt1", bufs=VT_BUFS))
                ot_pool = ctx.enter_context(tc.tile_pool(name="ot1", bufs=OT_BUFS))
                et_pool = ctx.enter_context(tc.tile_pool(name="et1", bufs=ET_BUFS))
                rr_pool = ctx.enter_context(tc.tile_pool(name="rr1", bufs=RR_BUFS))
                proj_ps = ctx.enter_context(tc.tile_pool(name="pps1", bufs=PROJ_BUFS, space="PSUM"))
                attn_ps = ctx.enter_context(tc.tile_pool(name="aps1", bufs=ATTN_BUFS, space="PSUM"))
                po_ps = ctx.enter_context(tc.tile_pool(name="pops1", bufs=PO_BUFS, space="PSUM"))
                pools = (qk_pool, vt_pool, ot_pool, et_pool, rr_pool, proj_ps, attn_ps, po_ps)

                for chunk in range(S // HC1):
                    q0 = chunk * HC1
                    src_t, stage_t = [], []
                    for cb in range(NCB):
                        cs = slice(cb * P, (cb + 1) * P)
                        t = src_pool.tile([P, HC1, S], bf, tag=f"src{cb}", name=f"src{cb}")
                        nc.sync.dma_start(out=t, in_=xtbf[cs, q0:q0 + HC1, :])
                        src_t.append(t)
                        # stage layout (hb, w, hi): contiguous runs on both
                        # DMA sides of the blocked write
                        st = stage_pool.tile([P, HB, HC1, HC2], f32, tag=f"stg{cb}", name=f"stg{cb}")
                        stage_t.append(st)
                    for g in range(HC1 // G):
                        s0 = g * G
                        gsl = slice(s0, s0 + G)
                        pods = attn_group(src_t, gsl, s0, wq, wk, wv, wo, pools)
                        for co in range(NCB):
                            # pods: (p, 4 w-seq, 128 h) -> stage (hb, w in gsl, hi)
                            nc.vector.tensor_copy(
                                stage_t[co][:, :, gsl, :].rearrange("p b q i -> p q b i"),
                                pods[co].rearrange("p (q b i) -> p q b i", q=G, b=HB))
                    for cb in range(NCB):
                        cs = slice(cb * P, (cb + 1) * P)
                        nc.sync.dma_start(
                            out=ohT2[:, cs, q0:q0 + HC1, :].rearrange("b c w i -> c b w i"),
                            in_=stage_t[cb])

        def width_pass():
            """Pass 2: width attention (seq along w, one per h).  h-chunk =
            hb block; out = xs + oh + ow in natural layout."""
            wq, wk, wv, wo = (w_sb["wq_w"], w_sb["wk_w"], w_sb["wv_w"], w_sb["wo_w"])
            with ExitStack() as ctx:
                src_pool = ctx.enter_context(tc.tile_pool(name="src2", bufs=2))
                resid_pool = ctx.enter_context(tc.tile_pool(name="res2", bufs=2))
                oh_pool = ctx.enter_context(tc.tile_pool(name="oh2", bufs=2))
                stage_pool = ctx.enter_context(tc.tile_pool(name="stg2", bufs=2))
                qk_pool = ctx.enter_context(tc.tile_pool(name="qk2", bufs=QK_BUFS))
                vt_pool = ctx.enter_context(tc.tile_pool(name="vt2", bufs=VT_BUFS))
                ot_pool = ctx.enter_context(tc.tile_pool(name="ot2", bufs=OT_BUFS))
                et_pool = ctx.enter_context(tc.tile_pool(name="et2", bufs=ET_BUFS))
                rr_pool = ctx.enter_context(tc.tile_pool(name="rr2", bufs=RR_BUFS))
                proj_ps = ctx.enter_context(tc.tile_pool(name="pps2", bufs=PROJ_BUFS, space="PSUM"))
                attn_ps = ctx.enter_context(tc.tile_pool(name="aps2", bufs=ATTN_BUFS, space="PSUM"))
                po_ps = ctx.enter_context(tc.tile_pool(name="pops2", bufs=PO_BUFS, space="PSUM"))
                pools = (qk_pool, vt_pool, ot_pool, et_pool, rr_pool, proj_ps, attn_ps, po_ps)

                for hb in range(HB):
                    q0 = hb * HC2
                    src_t, resid_t, stage_t = [], [], []
                    for cb in range(NCB):
                        cs = slice(cb * P, (cb + 1) * P)
                        t = src_pool.tile([P, HC2, S], bf, tag=f"src{cb}", name=f"src{cb}")
                        nc.sync.dma_start(out=t, in_=xbf[cs, q0:q0 + HC2, :])
                        src_t.append(t)
                        rt = resid_pool.tile([P, HC2, S], f32, tag=f"res{cb}", name=f"res{cb}")
                        nc.sync.dma_start(out=rt, in_=xf[cs, q0:q0 + HC2, :])
                        resid_t.append(rt)
                        oht = oh_pool.tile([P, S, HC2], f32, tag=f"oh{cb}", name=f"oh{cb}")
                        nc.sync.dma_start(out=oht, in_=ohT2[hb, cs, :, :])
                        # fold oh into the residual once per chunk
                        nc.gpsimd.tensor_tensor(
                            out=rt, in0=rt,
                            in1=oht.rearrange("p w i -> p i w"),
                            op=mybir.AluOpType.add)
                        st = stage_pool.tile([P, HC2, S], f32, tag=f"stg{cb}", name=f"stg{cb}")
                        stage_t.append(st)
                    for g in range(HC2 // G):
                        s0 = g * G
                        gsl = slice(s0, s0 + G)
                        pods = attn_group(src_t, gsl, s0, wq, wk, wv, wo, pools)
                        for co in range(NCB):
                            nc.vector.tensor_add(
                                stage_t[co][:, gsl, :],
                                pods[co].rearrange("p (q s) -> p q s", q=G),
                                resid_t[co][:, gsl, :])
                    for cb in range(NCB):
                        cs = slice(cb * P, (cb + 1) * P)
                        nc.sync.dma_start(out=out[cs, q0:q0 + HC2, :], in_=stage_t[cb])

        height_pass()
        width_pass()

    nc.compile()
    return nc


def _get_program():
    global _PROG
    if _PROG is None:
        _PROG = _build_program()
    return _PROG


def kernel(xs, Wq_h, Wk_h, Wv_h, Wo_h, Wq_w, Wk_w, Wv_w, Wo_w):
    from concourse.bass_utils import run_bass_kernel_spmd

    nc = _get_program()

    wmap = {
        "wq_w": Wq_w, "wk_w": Wk_w, "wv_w": Wv_w, "wo_w": Wo_w,
        "wq_h": Wq_h, "wk_h": Wk_h, "wv_h": Wv_h, "wo_h": Wo_h,
    }
    wt_np = {n: np.ascontiguousarray(np.asarray(w, dtype=np.float32).T).astype(_BF16)
             for n, w in wmap.items()}

    xs = np.asarray(xs, dtype=np.float32)
    in_maps = []
    for b in range(NCORES):
        xb = np.ascontiguousarray(xs[b])                        # (C, H, W) f32
        xbf = xb.astype(_BF16)                                  # (C, H, W) bf16
        xtbf = np.ascontiguousarray(np.swapaxes(xb, 1, 2)).astype(_BF16)  # (C, W, H)
        in_maps.append({"xf": xb, "xbf": xbf, "xtbf": xtbf, **wt_np})

    res = run_bass_kernel_spmd(nc, in_maps, core_ids=list(range(NCORES)))
    return np.stack([res.results[b]["out"] for b in range(NCORES)], axis=0)
